# revision 4
# baseline (speedup 1.0000x reference)
"""Joint soft-histogram kernel for Trainium2 (Bass/Tile), 8-core data parallel.

Math (per batch b, K=256, L=1/256, W=L/2.5, N=65536 pixels):
    phi_k(x) = S_k(x) - S_{k+1}(x),   S_k(x) = sigmoid(640*x - 2.5*k)
    out[k, j] = sum_n phi_k(x_n) * phi_j(y_n) / N

v8 structure ("Phi-x"): out = Dcol(M') / N with M' = Phi_x^T Sy (256 x 257),
Phi_x[k, n] = phi_k(x_n). The x-side row difference is applied per chunk on
DVE (fp16 packed, 2x mode, ~2.1us/group) BEFORE the matmul, so:
  - lhsT has exactly 256 rows -> 2 matmuls per chunk, NO tail matmul
    (v7's 512 one-row tail matmuls each cost a full 257-col stream = 1/3 of
    PE time).
  - M' entries stay O(256) (sum_n phi <= ~290) instead of O(N), so fp32 PSUM
    accumulates all 512 chunks in ONE chain -- no segment drains, no SBUF
    accumulators, no bidiagonal epilogue matmuls.
  - epilogue: one DVE column-diff from PSUM + one ACT scale by 1/N + one DMA.

Engine plan: ACT ~240us (staged sigmoid, the floor: ACT is the only engine
with transcendentals, 1 elem/cycle/lane @1.2GHz), DVE = preadd share + phi
diff, GPSIMD = preadd share, PE ~190us (1024 matmuls x 257 cols).
Preadd A[p, c*KP+j] = 640*v[p,c] - 2.5*j runs as broadcast-AP tensor_tensor
on DVE (4.4us/group) / GPSIMD (14.4us/group) / fused per-chunk
ACTIVATE-with-bias on ScalarE ('a', no preadd at all), split per the knobs
below to balance the three engines.

Sharding: pure data parallel, batch b -> core b.
"""

import numpy as np

import concourse.bass as bass
import concourse.tile as tile
from concourse import bacc, mybir
from concourse.bass_utils import run_bass_kernel_spmd

F32 = mybir.dt.float32
F16 = mybir.dt.float16

B = 8
K = 256
KB = K + 1            # 257 sigmoid taps per side (k = 0..256)
KP = K + 2            # 258: per-chunk stride in staged tiles (even)
NPIX = 65536
NCHUNK = 512
XG = 16               # chunks per staged group
NG = NCHUNK // XG     # 32 groups
INV_N = 1.0 / NPIX

# --- tuning knobs -----------------------------------------------------------
# Preadd engine per (group, side): 'v' = DVE broadcast-TT, 'g' = GPSIMD TT,
# 'a' = per-chunk fused ACTIVATE with per-partition bias (no preadd at all).
# Balance target: ACT ~= GPSIMD ~= DVE ~= 250us
#   ACT = 238 + 2.8*n_a ; GPSIMD = 14.4*n_g ; DVE = 351 - 4.4*(n_g + n_a)
X_ENG = [('g' if g % 4 == 2 else 'v') for g in range(NG)]          # 8 g
Y_ENG = [('g' if g % 4 == 2 else ('a' if g % 8 == 0 else 'v'))
         for g in range(NG)]                                       # 8 g, 4 a
Y_ENG[12] = 'a'
# ---------------------------------------------------------------------------

_cached_nc = None


def _build():
    nc = bacc.Bacc("TRN2")
    xd = nc.declare_dram_parameter("x", [128, 512], F32, isOutput=False)
    yd = nc.declare_dram_parameter("y", [128, 512], F32, isOutput=False)
    kd = nc.declare_dram_parameter("krow", [128, KP], F32, isOutput=False)
    od = nc.declare_dram_parameter("out", [256, 256], F32, isOutput=True)

    sig = mybir.ActivationFunctionType.Sigmoid
    add = mybir.AluOpType.add

    with tile.TileContext(nc) as tc:
        with (
            tc.tile_pool(name="singles", bufs=1) as singles,
            tc.tile_pool(name="stage32", bufs=3) as stage32,
            tc.tile_pool(name="stage16", bufs=4) as stage16,
            tc.tile_pool(name="phi16", bufs=3) as phi16,
            tc.tile_pool(name="work", bufs=3) as work,
            tc.tile_pool(name="psum", bufs=1, space="PSUM") as psum,
        ):
            # Preload the sigmoid ACT table-set (~2.7us) while DMAs run:
            # memset a tiny tile, then a 1-wide dummy sigmoid.
            warm = singles.tile([128, 2], F32)
            nc.vector.memset(warm, 0.0)
            nc.scalar.activation(out=warm, in_=warm, func=sig)

            kr = singles.tile([128, KP], F32)
            nc.sync.dma_start(out=kr, in_=kd[:, :])
            xt = singles.tile([128, 512], F32)
            nc.sync.dma_start(out=xt, in_=xd[:, :])
            yt = singles.tile([128, 512], F32)
            nc.sync.dma_start(out=yt, in_=yd[:, :])

            # PSUM: M' accumulator, rows 0..127 (h=0) and 128..255 (h=1).
            # Entries stay O(256), so one fp32 chain over all 512 chunks is
            # numerically fine (roundoff ~3e-4 abs vs out*N scale ~10).
            Mp = psum.tile([128, 2, 512], F32, tag="mp")

            def preadd_sigmoid(src, g, eng, tag, pieces=1):
                # pieces>1 splits the preadd+sigmoid into smaller units so
                # the first matmuls can start sooner (startup ramp).
                a = stage32.tile([128, XG, KP], F32, tag="a" + tag)
                s = stage16.tile([128, XG, KP], F16, tag="s" + tag)
                tt = nc.gpsimd.tensor_tensor if eng == 'g' else \
                    nc.vector.tensor_tensor
                w = XG // pieces
                for p in range(pieces):
                    lo, hi = p * w, (p + 1) * w
                    tt(
                        out=a[:, lo:hi, :],
                        in0=src[:, g * XG + lo:g * XG + hi].unsqueeze(2)
                            .broadcast_to([128, w, KP]),
                        in1=kr.unsqueeze(1).broadcast_to([128, w, KP]),
                        op=add,
                    )
                    nc.scalar.activation(
                        out=s[:, lo:hi, :], in_=a[:, lo:hi, :], func=sig,
                    )
                return s

            for g in range(NG):
                npc = 4 if g == 0 else (2 if g == 1 else 1)
                sx = preadd_sigmoid(xt, g, X_ENG[g], "x", pieces=npc)
                # Phi_x = S[j] - S[j+1], fp16 packed SBUF -> DVE 2x mode.
                px = phi16.tile([128, XG, K], F16, tag="px")
                w = XG // npc
                for p in range(npc):
                    lo, hi = p * w, (p + 1) * w
                    nc.vector.tensor_sub(
                        out=px[:, lo:hi, :],
                        in0=sx[:, lo:hi, 0:K],
                        in1=sx[:, lo:hi, 1:KB],
                    )
                fused_y = Y_ENG[g] == 'a'
                if not fused_y:
                    sy = preadd_sigmoid(yt, g, Y_ENG[g], "y", pieces=npc)
                for i in range(XG):
                    c = g * XG + i
                    first = c == 0
                    last = c == NCHUNK - 1
                    if fused_y:
                        tyt = work.tile([128, KB], F16, tag="tyf")
                        nc.scalar.activation(
                            out=tyt, in_=kr[:, 0:KB], func=sig,
                            bias=yt[:, c:c + 1], scale=1.0,
                        )
                        ty = tyt[:, :]
                    else:
                        ty = sy[:, i, 0:KB]
                    nc.tensor.matmul(
                        Mp[:, 0, 0:KB],
                        lhsT=px[:, i, 0:128],
                        rhs=ty,
                        start=first,
                        stop=last,
                    )
                    nc.tensor.matmul(
                        Mp[:, 1, 0:KB],
                        lhsT=px[:, i, 128:256],
                        rhs=ty,
                        start=first,
                        stop=last,
                    )

            # Epilogue: out[k, j] = (M'[k, j] - M'[k, j+1]) / N. TT cannot
            # read two PSUM operands, so ACT first copies+scales M' to SBUF.
            mc = work.tile([128, 2, KB], F32, tag="epc")
            nc.scalar.mul(mc, Mp[:, :, 0:KB], INV_N)
            t2 = work.tile([128, 2, K], F32, tag="ep2")
            nc.vector.tensor_sub(
                out=t2, in0=mc[:, :, 0:K], in1=mc[:, :, 1:KB],
            )
            od_r = od.rearrange("(h p) j -> p h j", h=2)
            nc.sync.dma_start(out=od_r, in_=t2)

    nc.finalize()
    return nc


def _get_nc():
    global _cached_nc
    if _cached_nc is None:
        _cached_nc = _build()
    return _cached_nc


def _krow():
    row = np.arange(KP, dtype=np.float32) * np.float32(-2.5)
    return np.tile(row[None, :], (128, 1))


def _in_maps(x, y):
    x = np.asarray(x, dtype=np.float32)
    y = np.asarray(y, dtype=np.float32)
    kr = _krow()
    maps = []
    for b in range(B):
        x6 = np.ascontiguousarray(x[b].reshape(128, 512) * np.float32(640.0))
        y6 = np.ascontiguousarray(y[b].reshape(128, 512) * np.float32(640.0))
        maps.append({"x": x6, "y": y6, "krow": kr})
    return maps


def run(x, y, trace=False, **trace_kw):
    """Run on all 8 cores; returns (out (8,256,256) f32, BassKernelResults)."""
    nc = _get_nc()
    res = run_bass_kernel_spmd(nc, _in_maps(x, y), list(range(B)), trace=trace,
                               **trace_kw)
    out = np.stack([res.results[b]["out"] for b in range(B)]).astype(np.float32)
    return out, res


def kernel(x, y):
    out, _ = run(x, y)
    return out
